# revision 14
# baseline (speedup 1.0000x reference)
"""Trainium2 Bass kernel for nn_MultiHeadAttention_523986010579.

Strategy: data-parallel over batch (B=32) across 8 NeuronCores (4 batches
per core).  Inside each core everything is computed with bf16 matmul
operands / fp32 PSUM accumulation.

Per-core dataflow (per batch b):
  qT,kT,vT fed pre-transposed (d_emb on partitions) from host.
  q1  = qT.T @ Wq'            (row-major, L on partitions; Wq' = Wq/sqrt(Dk))
  k1  = kT.T @ Wk
  v1  = vT.T @ Wv             (row-major)
  q1cT[dk,o] = (q1.T @ Wconv.T + bconv') * (1-mask1)[o]   (bias via K=1 matmul,
  k1cT[dk,o] = (k1.T @ Wconv.T + bconv ) * (1-mask2)[o]    mask folded at evac)
  per head h:
    logitsT[k,q] = k1cT_h.T @ q1cT_h          (K=64 contraction)
    expT = exp(logitsT)                        (no max-sub: |logits| < 0.2)
    sums[q] = ones.T @ expT                    (PE ones-trick)
    out2T[dv,q] = (v1_h.T @ expT) * (1/sums)[q]  (recip broadcast via K=1 matmul)
  out3 = out2T.T @ Wfc + q                    (residual, fp32)
  LayerNorm(out3) on the free axis            (fp32)
Outputs: unnormalized expT (bf16) + sums (f32) + normalized out (f32).
Host: attn = expT.T / sums, out = out*gamma+beta, nan->0, gather shards.

The mask fill value 1e-9 in the reference is numerically ~0, so masking
is exactly a {0,1} row/col scale of the logits (error ~1e-9, far below
fp32 noise).
"""

import numpy as np
import ml_dtypes

B, L, D, DK, DV, H = 32, 512, 768, 512, 1024, 8
NCORES = 8
BC = B // NCORES          # batches per core
KC_D = D // 128           # 6  k-chunks of d_emb
MC_L = L // 128           # 4  m-chunks of L
KC_L = L // 128           # 4  k-chunks of L (conv contraction / attn keys)
MC_DK = DK // 128         # 4  m-chunks of d_k
HC_DV = DV // 128         # 8  chunks of d_v (== heads, dh_v = 128)
DH_K = DK // H            # 64
LN_EPS = 1e-6

bf16 = ml_dtypes.bfloat16

_CACHE = {}


def _build_program(bc=BC):
    import concourse.bass as bass
    import concourse.bacc as bacc
    import concourse.tile as tile
    import concourse.mybir as mybir
    from contextlib import ExitStack

    dt = mybir.dt
    ALU = mybir.AluOpType
    ACTF = mybir.ActivationFunctionType

    global BC
    BC_saved = BC
    BC = bc
    nc = bacc.Bacc("TRN2", target_bir_lowering=False, debug=False)

    # ---- DRAM I/O (per-core shard shapes) ----
    din = {}
    def dram_in(name, shape, dtype):
        din[name] = nc.dram_tensor(name, list(shape), dtype, kind="ExternalInput").ap()
        return din[name]

    qT_d = dram_in("qT", (BC, D, L), dt.bfloat16)
    kT_d = dram_in("kT", (BC, D, L), dt.bfloat16)
    vT_d = dram_in("vT", (BC, D, L), dt.bfloat16)
    qrow_d = dram_in("qrow", (BC, L, D), dt.float32)
    wq_d = dram_in("wq", (D, DK), dt.bfloat16)      # pre-scaled by 1/sqrt(DK)
    wk_d = dram_in("wk", (D, DK), dt.bfloat16)
    wv_d = dram_in("wv", (D, DV), dt.bfloat16)
    wct_d = dram_in("wct", (L, L), dt.bfloat16)     # Wconv.T
    wfc_d = dram_in("wfc", (DV, D), dt.bfloat16)
    bq_d = dram_in("bq", (1, L), dt.bfloat16)       # bconv/sqrt(DK)
    bk_d = dram_in("bk", (1, L), dt.bfloat16)       # bconv
    c1_d = dram_in("c1", (1, BC * L), dt.bfloat16)   # 1-mask1 per batch
    c2_d = dram_in("c2", (1, BC * L), dt.bfloat16)   # 1-mask2 per batch

    expT_o = nc.dram_tensor("expT_o", [BC, H, L, L], dt.bfloat16, kind="ExternalOutput").ap()
    sums_o = nc.dram_tensor("sums_o", [BC, H * L], dt.float32, kind="ExternalOutput").ap()
    out_o = nc.dram_tensor("out_o", [BC, L, D], dt.float32, kind="ExternalOutput").ap()

    with tile.TileContext(nc) as tc, ExitStack() as ctx:
        wp = ctx.enter_context(tc.tile_pool(name="wp", bufs=1))
        io = ctx.enter_context(tc.tile_pool(name="io", bufs=2))
        ip = ctx.enter_context(tc.tile_pool(name="ip", bufs=2))
        sp = ctx.enter_context(tc.tile_pool(name="sp", bufs=4))
        pp = ctx.enter_context(tc.tile_pool(name="pp", bufs=5, space="PSUM"))

        # ---- weights / constants (once) ----
        wq = wp.tile([128, KC_D * DK], dt.bfloat16, name="wq_t", tag="wq_t")
        nc.sync.dma_start(wq[:].rearrange("p (kc j) -> p kc j", kc=KC_D),
                          wq_d.rearrange("(kc p) j -> p kc j", p=128))
        wk = wp.tile([128, KC_D * DK], dt.bfloat16, name="wk_t", tag="wk_t")
        nc.sync.dma_start(wk[:].rearrange("p (kc j) -> p kc j", kc=KC_D),
                          wk_d.rearrange("(kc p) j -> p kc j", p=128))
        wv = wp.tile([128, KC_D * DV], dt.bfloat16, name="wv_t", tag="wv_t")
        nc.sync.dma_start(wv[:].rearrange("p (kc j) -> p kc j", kc=KC_D),
                          wv_d.rearrange("(kc p) j -> p kc j", p=128))
        wct = wp.tile([128, KC_L * L], dt.bfloat16, name="wct_t", tag="wct_t")
        nc.sync.dma_start(wct[:].rearrange("p (kc j) -> p kc j", kc=KC_L),
                          wct_d.rearrange("(kc p) j -> p kc j", p=128))
        wfc = wp.tile([128, HC_DV * D], dt.bfloat16, name="wfc_t", tag="wfc_t")
        nc.sync.dma_start(wfc[:].rearrange("p (kc j) -> p kc j", kc=HC_DV),
                          wfc_d.rearrange("(kc p) j -> p kc j", p=128))
        bq = wp.tile([1, L], dt.bfloat16, name="bq_t", tag="bq_t")
        nc.sync.dma_start(bq[:], bq_d)
        bk = wp.tile([1, L], dt.bfloat16, name="bk_t", tag="bk_t")
        nc.sync.dma_start(bk[:], bk_d)
        c1a = wp.tile([1, BC * L], dt.bfloat16, name="c1a_t", tag="c1a_t")
        nc.sync.dma_start(c1a[:], c1_d)
        c2a = wp.tile([1, BC * L], dt.bfloat16, name="c2a_t", tag="c2a_t")
        nc.sync.dma_start(c2a[:], c2_d)
        ones_col = wp.tile([128, 1], dt.bfloat16, name="ones_col", tag="ones_col")
        nc.vector.memset(ones_col[:], 1.0)
        ones_row = wp.tile([1, 128], dt.bfloat16, name="ones_row", tag="ones_row")
        nc.vector.memset(ones_row[:], 1.0)
        eps_col = wp.tile([128, 1], dt.float32, name="eps_col", tag="eps_col")
        nc.vector.memset(eps_col[:], float(LN_EPS))

        for b in range(BC):
            # ---- load activations ----
            qT = io.tile([128, KC_D * L], dt.bfloat16, name="qT_t", tag="qT_t")
            nc.sync.dma_start(qT[:].rearrange("p (kc j) -> p kc j", kc=KC_D),
                              qT_d[b].rearrange("(kc p) j -> p kc j", p=128))
            kT = io.tile([128, KC_D * L], dt.bfloat16, name="kT_t", tag="kT_t")
            nc.sync.dma_start(kT[:].rearrange("p (kc j) -> p kc j", kc=KC_D),
                              kT_d[b].rearrange("(kc p) j -> p kc j", p=128))
            vT = io.tile([128, KC_D * L], dt.bfloat16, name="vT_t", tag="vT_t")
            nc.sync.dma_start(vT[:].rearrange("p (kc j) -> p kc j", kc=KC_D),
                              vT_d[b].rearrange("(kc p) j -> p kc j", p=128))

            # ---- mask broadcast tiles: ones_row.T @ c_row ----
            c1b = ip.tile([128, L], dt.float32, name="c1b_t", tag="c1b_t", bufs=1)
            c2b = ip.tile([128, L], dt.float32, name="c2b_t", tag="c2b_t", bufs=1)
            for cb, ca in ((c1b, c1a), (c2b, c2a)):
                ps = pp.tile([128, 512], dt.float32, name="ps_cb", tag="pS", bufs=3)
                nc.tensor.matmul(ps[:], ones_row[:], ca[0:1, b * L:(b + 1) * L],
                                 start=True, stop=True)
                nc.scalar.copy(cb[:], ps[:])

            # ---- projections: q1/k1 row-major ----
            q1 = ip.tile([128, MC_L * DK], dt.bfloat16, name="q1_t", tag="q1_t", bufs=1)
            k1 = ip.tile([128, MC_L * DK], dt.bfloat16, name="k1_t", tag="k1_t", bufs=1)
            for dst, src, w in ((q1, qT, wq), (k1, kT, wk)):
                for m in range(MC_L):
                    ps = pp.tile([128, 512], dt.float32, name="ps_p", tag="pA")
                    for kc in range(KC_D):
                        nc.tensor.matmul(
                            ps[:],
                            src[:, kc * L + m * 128: kc * L + (m + 1) * 128],
                            w[:, kc * DK:(kc + 1) * DK],
                            start=(kc == 0), stop=(kc == KC_D - 1))
                    nc.vector.tensor_copy(dst[:, m * DK:(m + 1) * DK], ps[:])

            # ---- v1 row-major (dv=1024 -> two 512 psums per m-chunk) ----
            v1 = ip.tile([128, MC_L * DV], dt.bfloat16, name="v1_t", tag="v1_t", bufs=2)
            for m in range(MC_L):
                for half in range(2):
                    ps = pp.tile([128, 512], dt.float32, name="ps_v", tag="pA")
                    for kc in range(KC_D):
                        nc.tensor.matmul(
                            ps[:],
                            vT[:, kc * L + m * 128: kc * L + (m + 1) * 128],
                            wv[:, kc * DV + half * 512: kc * DV + (half + 1) * 512],
                            start=(kc == 0), stop=(kc == KC_D - 1))
                    nc.scalar.copy(v1[:, m * DV + half * 512: m * DV + (half + 1) * 512], ps[:])

            # ---- conv(+bias) then mask at evac -> q1cT / k1cT ----
            q1cT = ip.tile([128, MC_DK * L], dt.bfloat16, name="q1cT_t", tag="q1cT_t")
            k1cT = ip.tile([128, MC_DK * L], dt.bfloat16, name="k1cT_t", tag="k1cT_t")
            for dst, src, bias, cb in ((q1cT, q1, bq, c1b), (k1cT, k1, bk, c2b)):
                for m in range(MC_DK):
                    ps = pp.tile([128, 512], dt.float32, name="ps_c", tag="pA")
                    for kc in range(KC_L):
                        nc.tensor.matmul(
                            ps[:],
                            src[:, kc * DK + m * 128: kc * DK + (m + 1) * 128],
                            wct[:, kc * L:(kc + 1) * L],
                            start=(kc == 0), stop=False)
                    nc.tensor.matmul(ps[:], ones_row[:], bias[:], start=False, stop=True)
                    nc.vector.tensor_tensor(
                        out=dst[:, m * L:(m + 1) * L], in0=ps[:], in1=cb[:], op=ALU.mult)

            # ---- attention heads ----
            o2T = ip.tile([128, HC_DV * L], dt.bfloat16, name="o2T_t", tag="o2T_t", bufs=2)
            for h in range(H):
                jc = h // 2
                d0 = (h % 2) * DH_K
                expT = sp.tile([128, KC_L * L], dt.bfloat16, name="expT_t", tag="expT_t", bufs=3)
                ps_s = pp.tile([1, 512], dt.float32, name="ps_s", tag="pS", bufs=3)
                for kc in range(KC_L):
                    ps_l = pp.tile([128, 512], dt.float32, name="ps_l", tag="pA")
                    nc.tensor.matmul(
                        ps_l[:],
                        k1cT[d0:d0 + DH_K, jc * L + kc * 128: jc * L + (kc + 1) * 128],
                        q1cT[d0:d0 + DH_K, jc * L:(jc + 1) * L],
                        start=True, stop=True,
                        tile_position=(d0, 0))
                    nc.scalar.activation(expT[:, kc * L:(kc + 1) * L], ps_l[:], ACTF.Exp)
                for kc in range(KC_L):
                    nc.tensor.matmul(ps_s[:], ones_col[:], expT[:, kc * L:(kc + 1) * L],
                                     start=(kc == 0), stop=(kc == KC_L - 1))
                # out2T_unnorm = v1_h.T @ expT  (keeps PE off the recip path)
                ps_o2 = pp.tile([128, 512], dt.float32, name="ps_o2", tag="pS", bufs=3)
                for kc in range(KC_L):
                    nc.tensor.matmul(
                        ps_o2[:],
                        v1[:, kc * DV + h * 128: kc * DV + (h + 1) * 128],
                        expT[:, kc * L:(kc + 1) * L],
                        start=(kc == 0), stop=(kc == KC_L - 1))
                sums_h = sp.tile([1, L], dt.float32, name="sums_h", tag="sums_h", bufs=2)
                nc.vector.tensor_copy(sums_h[:], ps_s[:])
                nc.sync.dma_start(sums_o[b:b + 1, h * L:(h + 1) * L], sums_h[:])
                nc.sync.dma_start(
                    expT_o[b, h].rearrange("(kc p) q -> p kc q", p=128),
                    expT[:].rearrange("p (kc q) -> p kc q", kc=KC_L))
                # reciprocal row -> broadcast tile (PE rb matmul is cheap, off critical path)
                recf = sp.tile([1, L], dt.float32, name="recf_t", tag="recf_t", bufs=2)
                nc.vector.reciprocal_approx_fast(out=recf[:], in_=ps_s[:])
                rb = sp.tile([128, L], dt.float32, name="rb_t", tag="rb_t", bufs=2)
                nc.gpsimd.partition_broadcast(rb[:], recf[:])
                nc.vector.tensor_tensor(out=o2T[:, h * L:(h + 1) * L],
                                        in0=ps_o2[:], in1=rb[:], op=ALU.mult)

            # ---- fc + residual + LayerNorm ----
            x = ip.tile([128, MC_L * D], dt.float32, name="x_t", tag="x_t", bufs=1)
            scratch = ip.tile([128, D], dt.bfloat16, name="scr_t", tag="scr_t", bufs=2)
            for m in range(MC_L):
                qrow = io.tile([128, D], dt.float32, name="qrow_t", tag="qrow_t", bufs=2)
                nc.sync.dma_start(qrow[:], qrow_d[b, m * 128:(m + 1) * 128, :])
                ps3a = pp.tile([128, 512], dt.float32, name="ps3a", tag="pA")
                ps3b = pp.tile([128, 512], dt.float32, name="ps3b", tag="pA")
                for hc in range(HC_DV):
                    nc.tensor.matmul(
                        ps3a[:],
                        o2T[:, hc * L + m * 128: hc * L + (m + 1) * 128],
                        wfc[:, hc * D: hc * D + 512],
                        start=(hc == 0), stop=(hc == HC_DV - 1))
                for hc in range(HC_DV):
                    nc.tensor.matmul(
                        ps3b[:, 0:256],
                        o2T[:, hc * L + m * 128: hc * L + (m + 1) * 128],
                        wfc[:, hc * D + 512: hc * D + D],
                        start=(hc == 0), stop=(hc == HC_DV - 1))
                s1a = sp.tile([128, 1], dt.float32, name="s1a_t", tag="s1a_t", bufs=4)
                s1b = sp.tile([128, 1], dt.float32, name="s1b_t", tag="s1b_t", bufs=4)
                s1 = sp.tile([128, 1], dt.float32, name="s1_t", tag="s1_t", bufs=4)
                xm = x[:, m * D:(m + 1) * D]
                nc.vector.scalar_tensor_tensor(
                    out=x[:, m * D: m * D + 512], in0=ps3a[:], scalar=0.0,
                    in1=qrow[:, 0:512],
                    op0=ALU.add, op1=ALU.add, accum_out=s1a[:])
                nc.vector.scalar_tensor_tensor(
                    out=x[:, m * D + 512:(m + 1) * D], in0=ps3b[:, 0:256], scalar=0.0,
                    in1=qrow[:, 512:D],
                    op0=ALU.add, op1=ALU.add, accum_out=s1b[:])
                nc.vector.tensor_tensor(out=s1[:], in0=s1a[:], in1=s1b[:], op=ALU.add)
                mun = sp.tile([128, 1], dt.float32, name="mun_t", tag="mun_t", bufs=4)
                nc.vector.tensor_scalar_mul(mun[:], s1[:], -1.0 / D)
                ssq = sp.tile([128, 1], dt.float32, name="ssq_t", tag="ssq_t", bufs=4)
                nc.scalar.activation(scratch[:], xm, ACTF.Square,
                                     bias=mun[:], scale=1.0, accum_out=ssq[:])
                std = sp.tile([128, 1], dt.float32, name="std_t", tag="std_t", bufs=4)
                nc.scalar.activation(std[:], ssq[:], ACTF.Sqrt,
                                     bias=eps_col[:], scale=1.0 / D)
                rstd = sp.tile([128, 1], dt.float32, name="rstd_t", tag="rstd_t", bufs=4)
                nc.vector.reciprocal_approx_fast(out=rstd[:], in_=std[:])
                nc.vector.tensor_scalar(out=xm, in0=xm, scalar1=mun[:], scalar2=rstd[:],
                                        op0=ALU.add, op1=ALU.mult)
            nc.sync.dma_start(out_o[b].rearrange("(m p) e -> p m e", p=128),
                              x[:].rearrange("p (m e) -> p m e", m=MC_L))

    nc.compile()
    globals()['BC'] = BC_saved
    return nc


def _get_nc():
    if "nc" not in _CACHE:
        _CACHE["nc"] = _build_program()
    return _CACHE["nc"]


def _prep_inputs(q, k, v, Wq, Wk, Wv, Wconv, bconv, Wfc, mask1, mask2):
    temp = np.sqrt(np.float32(DK))
    wq = (np.asarray(Wq, np.float32) / temp).astype(bf16)
    wk = np.asarray(Wk, np.float32).astype(bf16)
    wv = np.asarray(Wv, np.float32).astype(bf16)
    wct = np.ascontiguousarray(np.asarray(Wconv, np.float32).T).astype(bf16)
    wfc = np.asarray(Wfc, np.float32).astype(bf16)
    bq = (np.asarray(bconv, np.float32) / temp).astype(bf16).reshape(1, L)
    bk = np.asarray(bconv, np.float32).astype(bf16).reshape(1, L)
    c1 = (1.0 - np.asarray(mask1, np.float32))
    c2 = (1.0 - np.asarray(mask2, np.float32))
    q = np.asarray(q, np.float32)
    k = np.asarray(k, np.float32)
    v = np.asarray(v, np.float32)
    qT = np.ascontiguousarray(q.transpose(0, 2, 1)).astype(bf16)
    kT = np.ascontiguousarray(k.transpose(0, 2, 1)).astype(bf16)
    vT = np.ascontiguousarray(v.transpose(0, 2, 1)).astype(bf16)

    in_maps = []
    for c in range(NCORES):
        s = slice(c * BC, (c + 1) * BC)
        in_maps.append({
            "qT": qT[s], "kT": kT[s], "vT": vT[s],
            "qrow": np.ascontiguousarray(q[s]),
            "wq": wq, "wk": wk, "wv": wv, "wct": wct, "wfc": wfc,
            "bq": bq, "bk": bk,
            "c1": np.ascontiguousarray(c1[s].reshape(1, BC * L)).astype(bf16),
            "c2": np.ascontiguousarray(c2[s].reshape(1, BC * L)).astype(bf16),
        })
    return in_maps


def _run(in_maps, trace=False):
    from concourse.bass_utils import run_bass_kernel_spmd
    nc = _get_nc()
    res = run_bass_kernel_spmd(nc, in_maps, list(range(NCORES)), trace=trace)
    return res


def _gather(res, gamma, beta):
    out = np.empty((B, L, D), np.float32)
    attn = np.empty((B, H, L, L), np.float32)
    for c in range(NCORES):
        r = res.results[c]
        sums = np.asarray(r["sums_o"], np.float32).reshape(BC, H, L)
        expT = np.asarray(r["expT_o"]).astype(np.float32)       # (BC,H,L,L) [k,q]
        attn[c * BC:(c + 1) * BC] = expT.transpose(0, 1, 3, 2) / sums[:, :, :, None]
        out[c * BC:(c + 1) * BC] = r["out_o"]
    out = out * np.asarray(gamma, np.float32) + np.asarray(beta, np.float32)
    out = np.where(np.isnan(out), np.float32(0), out)
    attn = np.where(np.isnan(attn), np.float32(0), attn)
    return out, attn


def kernel(q, k, v, Wq, Wk, Wv, Wconv, bconv, Wfc, gamma, beta, mask1, mask2):
    in_maps = _prep_inputs(q, k, v, Wq, Wk, Wv, Wconv, bconv, Wfc, mask1, mask2)
    res = _run(in_maps, trace=False)
    return _gather(res, gamma, beta)


# revision 15
# speedup vs baseline: 1.1760x; 1.1760x over previous
"""Trainium2 Bass kernel for nn_MultiHeadAttention_523986010579.

Strategy: data-parallel over batch (B=32) across 8 NeuronCores (4 batches
per core).  Inside each core everything is computed with bf16 matmul
operands / fp32 PSUM accumulation.

Per-core dataflow (per batch b):
  qT,kT,vT fed pre-transposed (d_emb on partitions) from host.
  q1  = qT.T @ Wq'            (row-major, L on partitions; Wq' = Wq/sqrt(Dk))
  k1  = kT.T @ Wk
  v1  = vT.T @ Wv             (row-major)
  q1cT[dk,o] = (q1.T @ Wconv.T + bconv') * (1-mask1)[o]   (bias via K=1 matmul,
  k1cT[dk,o] = (k1.T @ Wconv.T + bconv ) * (1-mask2)[o]    mask folded at evac)
  per head h:
    logitsT[k,q] = k1cT_h.T @ q1cT_h          (K=64 contraction)
    expT = exp(logitsT)                        (no max-sub: |logits| < 0.2)
    sums[q] = ones.T @ expT                    (PE ones-trick)
    out2T[dv,q] = (v1_h.T @ expT) * (1/sums)[q]  (recip broadcast via K=1 matmul)
  out3 = out2T.T @ Wfc + q                    (residual, fp32)
  LayerNorm(out3) on the free axis            (fp32)
Outputs: unnormalized expT (bf16) + sums (f32) + normalized out (f32).
Host: attn = expT.T / sums, out = out*gamma+beta, nan->0, gather shards.

The mask fill value 1e-9 in the reference is numerically ~0, so masking
is exactly a {0,1} row/col scale of the logits (error ~1e-9, far below
fp32 noise).
"""

import numpy as np
import ml_dtypes

B, L, D, DK, DV, H = 32, 512, 768, 512, 1024, 8
NCORES = 8
BC = B // NCORES          # batches per core
KC_D = D // 128           # 6  k-chunks of d_emb
MC_L = L // 128           # 4  m-chunks of L
KC_L = L // 128           # 4  k-chunks of L (conv contraction / attn keys)
MC_DK = DK // 128         # 4  m-chunks of d_k
HC_DV = DV // 128         # 8  chunks of d_v (== heads, dh_v = 128)
DH_K = DK // H            # 64
LN_EPS = 1e-6

bf16 = ml_dtypes.bfloat16

_CACHE = {}


def _build_program(bc=BC):
    import concourse.bass as bass
    import concourse.bacc as bacc
    import concourse.tile as tile
    import concourse.mybir as mybir
    from contextlib import ExitStack

    dt = mybir.dt
    ALU = mybir.AluOpType
    ACTF = mybir.ActivationFunctionType

    global BC
    BC_saved = BC
    BC = bc
    nc = bacc.Bacc("TRN2", target_bir_lowering=False, debug=False)

    # ---- DRAM I/O (per-core shard shapes) ----
    din = {}
    def dram_in(name, shape, dtype):
        din[name] = nc.dram_tensor(name, list(shape), dtype, kind="ExternalInput").ap()
        return din[name]

    qT_d = dram_in("qT", (BC, D, L), dt.bfloat16)
    kT_d = dram_in("kT", (BC, D, L), dt.bfloat16)
    vT_d = dram_in("vT", (BC, D, L), dt.bfloat16)
    qrow_d = dram_in("qrow", (BC, L, D), dt.float32)
    wq_d = dram_in("wq", (D, DK), dt.bfloat16)      # pre-scaled by 1/sqrt(DK)
    wk_d = dram_in("wk", (D, DK), dt.bfloat16)
    wv_d = dram_in("wv", (D, DV), dt.bfloat16)
    wct_d = dram_in("wct", (L, L), dt.bfloat16)     # Wconv.T
    wfc_d = dram_in("wfc", (DV, D), dt.bfloat16)
    bq_d = dram_in("bq", (1, L), dt.bfloat16)       # bconv/sqrt(DK)
    bk_d = dram_in("bk", (1, L), dt.bfloat16)       # bconv
    c1_d = dram_in("c1", (1, BC * L), dt.bfloat16)   # 1-mask1 per batch
    c2_d = dram_in("c2", (1, BC * L), dt.bfloat16)   # 1-mask2 per batch

    expT_o = nc.dram_tensor("expT_o", [BC, H, L, L], dt.bfloat16, kind="ExternalOutput").ap()
    sums_o = nc.dram_tensor("sums_o", [BC, H * L], dt.float32, kind="ExternalOutput").ap()
    out_o = nc.dram_tensor("out_o", [BC, L, D], dt.float32, kind="ExternalOutput").ap()

    with tile.TileContext(nc) as tc, ExitStack() as ctx:
        wp = ctx.enter_context(tc.tile_pool(name="wp", bufs=1))
        io = ctx.enter_context(tc.tile_pool(name="io", bufs=2))
        ip = ctx.enter_context(tc.tile_pool(name="ip", bufs=2))
        sp = ctx.enter_context(tc.tile_pool(name="sp", bufs=4))
        pp = ctx.enter_context(tc.tile_pool(name="pp", bufs=5, space="PSUM"))

        # ---- weights / constants (once) ----
        wq = wp.tile([128, KC_D * DK], dt.bfloat16, name="wq_t", tag="wq_t")
        nc.sync.dma_start(wq[:].rearrange("p (kc j) -> p kc j", kc=KC_D),
                          wq_d.rearrange("(kc p) j -> p kc j", p=128))
        wk = wp.tile([128, KC_D * DK], dt.bfloat16, name="wk_t", tag="wk_t")
        nc.sync.dma_start(wk[:].rearrange("p (kc j) -> p kc j", kc=KC_D),
                          wk_d.rearrange("(kc p) j -> p kc j", p=128))
        wv = wp.tile([128, KC_D * DV], dt.bfloat16, name="wv_t", tag="wv_t")
        nc.sync.dma_start(wv[:].rearrange("p (kc j) -> p kc j", kc=KC_D),
                          wv_d.rearrange("(kc p) j -> p kc j", p=128))
        wct = wp.tile([128, KC_L * L], dt.bfloat16, name="wct_t", tag="wct_t")
        nc.sync.dma_start(wct[:].rearrange("p (kc j) -> p kc j", kc=KC_L),
                          wct_d.rearrange("(kc p) j -> p kc j", p=128))
        wfc = wp.tile([128, HC_DV * D], dt.bfloat16, name="wfc_t", tag="wfc_t")
        nc.sync.dma_start(wfc[:].rearrange("p (kc j) -> p kc j", kc=HC_DV),
                          wfc_d.rearrange("(kc p) j -> p kc j", p=128))
        bq = wp.tile([1, L], dt.bfloat16, name="bq_t", tag="bq_t")
        nc.sync.dma_start(bq[:], bq_d)
        bk = wp.tile([1, L], dt.bfloat16, name="bk_t", tag="bk_t")
        nc.sync.dma_start(bk[:], bk_d)
        c1a = wp.tile([1, BC * L], dt.bfloat16, name="c1a_t", tag="c1a_t")
        nc.sync.dma_start(c1a[:], c1_d)
        c2a = wp.tile([1, BC * L], dt.bfloat16, name="c2a_t", tag="c2a_t")
        nc.sync.dma_start(c2a[:], c2_d)
        ones_col = wp.tile([128, 1], dt.bfloat16, name="ones_col", tag="ones_col")
        nc.vector.memset(ones_col[:], 1.0)
        ones_row = wp.tile([1, 128], dt.bfloat16, name="ones_row", tag="ones_row")
        nc.vector.memset(ones_row[:], 1.0)
        eps_col = wp.tile([128, 1], dt.float32, name="eps_col", tag="eps_col")
        nc.vector.memset(eps_col[:], float(LN_EPS))

        for b in range(BC):
            # ---- load activations ----
            qT = io.tile([128, KC_D * L], dt.bfloat16, name="qT_t", tag="qT_t")
            nc.sync.dma_start(qT[:].rearrange("p (kc j) -> p kc j", kc=KC_D),
                              qT_d[b].rearrange("(kc p) j -> p kc j", p=128))
            kT = io.tile([128, KC_D * L], dt.bfloat16, name="kT_t", tag="kT_t")
            nc.sync.dma_start(kT[:].rearrange("p (kc j) -> p kc j", kc=KC_D),
                              kT_d[b].rearrange("(kc p) j -> p kc j", p=128))
            vT = io.tile([128, KC_D * L], dt.bfloat16, name="vT_t", tag="vT_t")
            nc.sync.dma_start(vT[:].rearrange("p (kc j) -> p kc j", kc=KC_D),
                              vT_d[b].rearrange("(kc p) j -> p kc j", p=128))

            # ---- mask broadcast tiles: ones_row.T @ c_row ----
            c1b = ip.tile([128, L], dt.float32, name="c1b_t", tag="c1b_t", bufs=1)
            c2b = ip.tile([128, L], dt.float32, name="c2b_t", tag="c2b_t", bufs=1)
            for cb, ca in ((c1b, c1a), (c2b, c2a)):
                ps = pp.tile([128, 512], dt.float32, name="ps_cb", tag="pS", bufs=3)
                nc.tensor.matmul(ps[:], ones_row[:], ca[0:1, b * L:(b + 1) * L],
                                 start=True, stop=True)
                nc.scalar.copy(cb[:], ps[:])

            # ---- projections: q1/k1 row-major ----
            q1 = ip.tile([128, MC_L * DK], dt.bfloat16, name="q1_t", tag="q1_t", bufs=1)
            k1 = ip.tile([128, MC_L * DK], dt.bfloat16, name="k1_t", tag="k1_t", bufs=1)
            for dst, src, w in ((q1, qT, wq), (k1, kT, wk)):
                for m in range(MC_L):
                    ps = pp.tile([128, 512], dt.float32, name="ps_p", tag="pA")
                    for kc in range(KC_D):
                        nc.tensor.matmul(
                            ps[:],
                            src[:, kc * L + m * 128: kc * L + (m + 1) * 128],
                            w[:, kc * DK:(kc + 1) * DK],
                            start=(kc == 0), stop=(kc == KC_D - 1))
                    nc.vector.tensor_copy(dst[:, m * DK:(m + 1) * DK], ps[:])

            # ---- v1 row-major (dv=1024 -> two 512 psums per m-chunk) ----
            v1 = ip.tile([128, MC_L * DV], dt.bfloat16, name="v1_t", tag="v1_t", bufs=2)
            for m in range(MC_L):
                for half in range(2):
                    ps = pp.tile([128, 512], dt.float32, name="ps_v", tag="pA")
                    for kc in range(KC_D):
                        nc.tensor.matmul(
                            ps[:],
                            vT[:, kc * L + m * 128: kc * L + (m + 1) * 128],
                            wv[:, kc * DV + half * 512: kc * DV + (half + 1) * 512],
                            start=(kc == 0), stop=(kc == KC_D - 1))
                    nc.scalar.copy(v1[:, m * DV + half * 512: m * DV + (half + 1) * 512], ps[:])

            # ---- conv(+bias) then mask at evac -> q1cT / k1cT ----
            q1cT = ip.tile([128, MC_DK * L], dt.bfloat16, name="q1cT_t", tag="q1cT_t")
            k1cT = ip.tile([128, MC_DK * L], dt.bfloat16, name="k1cT_t", tag="k1cT_t")
            for dst, src, bias, cb in ((q1cT, q1, bq, c1b), (k1cT, k1, bk, c2b)):
                for m in range(MC_DK):
                    ps = pp.tile([128, 512], dt.float32, name="ps_c", tag="pA")
                    for kc in range(KC_L):
                        nc.tensor.matmul(
                            ps[:],
                            src[:, kc * DK + m * 128: kc * DK + (m + 1) * 128],
                            wct[:, kc * L:(kc + 1) * L],
                            start=(kc == 0), stop=False)
                    nc.tensor.matmul(ps[:], ones_row[:], bias[:], start=False, stop=True)
                    nc.vector.tensor_tensor(
                        out=dst[:, m * L:(m + 1) * L], in0=ps[:], in1=cb[:], op=ALU.mult)

            # ---- attention heads ----
            o2T = ip.tile([128, HC_DV * L], dt.bfloat16, name="o2T_t", tag="o2T_t", bufs=2)
            for h in range(H):
                jc = h // 2
                d0 = (h % 2) * DH_K
                expT = sp.tile([128, KC_L * L], dt.bfloat16, name="expT_t", tag="expT_t", bufs=3)
                ps_s = pp.tile([1, 512], dt.float32, name="ps_s", tag="pS", bufs=3)
                for kc in range(KC_L):
                    ps_l = pp.tile([128, 512], dt.float32, name="ps_l", tag="pA")
                    nc.tensor.matmul(
                        ps_l[:],
                        k1cT[d0:d0 + DH_K, jc * L + kc * 128: jc * L + (kc + 1) * 128],
                        q1cT[d0:d0 + DH_K, jc * L:(jc + 1) * L],
                        start=True, stop=True)
                    nc.scalar.activation(expT[:, kc * L:(kc + 1) * L], ps_l[:], ACTF.Exp)
                for kc in range(KC_L):
                    nc.tensor.matmul(ps_s[:], ones_col[:], expT[:, kc * L:(kc + 1) * L],
                                     start=(kc == 0), stop=(kc == KC_L - 1))
                # out2T_unnorm = v1_h.T @ expT  (keeps PE off the recip path)
                ps_o2 = pp.tile([128, 512], dt.float32, name="ps_o2", tag="pS", bufs=3)
                for kc in range(KC_L):
                    nc.tensor.matmul(
                        ps_o2[:],
                        v1[:, kc * DV + h * 128: kc * DV + (h + 1) * 128],
                        expT[:, kc * L:(kc + 1) * L],
                        start=(kc == 0), stop=(kc == KC_L - 1))
                sums_h = sp.tile([1, L], dt.float32, name="sums_h", tag="sums_h", bufs=2)
                nc.vector.tensor_copy(sums_h[:], ps_s[:])
                nc.sync.dma_start(sums_o[b:b + 1, h * L:(h + 1) * L], sums_h[:])
                nc.sync.dma_start(
                    expT_o[b, h].rearrange("(kc p) q -> p kc q", p=128),
                    expT[:].rearrange("p (kc q) -> p kc q", kc=KC_L))
                # reciprocal row -> broadcast tile (PE rb matmul is cheap, off critical path)
                recf = sp.tile([1, L], dt.float32, name="recf_t", tag="recf_t", bufs=2)
                nc.vector.reciprocal_approx_fast(out=recf[:], in_=ps_s[:])
                rb = sp.tile([128, L], dt.float32, name="rb_t", tag="rb_t", bufs=2)
                nc.gpsimd.partition_broadcast(rb[:], recf[:])
                nc.vector.tensor_tensor(out=o2T[:, h * L:(h + 1) * L],
                                        in0=ps_o2[:], in1=rb[:], op=ALU.mult)

            # ---- fc + residual + LayerNorm ----
            x = ip.tile([128, MC_L * D], dt.float32, name="x_t", tag="x_t", bufs=1)
            scratch = ip.tile([128, D], dt.bfloat16, name="scr_t", tag="scr_t", bufs=2)
            for m in range(MC_L):
                qrow = io.tile([128, D], dt.float32, name="qrow_t", tag="qrow_t", bufs=2)
                nc.sync.dma_start(qrow[:], qrow_d[b, m * 128:(m + 1) * 128, :])
                ps3a = pp.tile([128, 512], dt.float32, name="ps3a", tag="pA")
                ps3b = pp.tile([128, 512], dt.float32, name="ps3b", tag="pA")
                for hc in range(HC_DV):
                    nc.tensor.matmul(
                        ps3a[:],
                        o2T[:, hc * L + m * 128: hc * L + (m + 1) * 128],
                        wfc[:, hc * D: hc * D + 512],
                        start=(hc == 0), stop=(hc == HC_DV - 1))
                for hc in range(HC_DV):
                    nc.tensor.matmul(
                        ps3b[:, 0:256],
                        o2T[:, hc * L + m * 128: hc * L + (m + 1) * 128],
                        wfc[:, hc * D + 512: hc * D + D],
                        start=(hc == 0), stop=(hc == HC_DV - 1))
                s1a = sp.tile([128, 1], dt.float32, name="s1a_t", tag="s1a_t", bufs=4)
                s1b = sp.tile([128, 1], dt.float32, name="s1b_t", tag="s1b_t", bufs=4)
                s1 = sp.tile([128, 1], dt.float32, name="s1_t", tag="s1_t", bufs=4)
                xm = x[:, m * D:(m + 1) * D]
                nc.vector.scalar_tensor_tensor(
                    out=x[:, m * D: m * D + 512], in0=ps3a[:], scalar=0.0,
                    in1=qrow[:, 0:512],
                    op0=ALU.add, op1=ALU.add, accum_out=s1a[:])
                nc.vector.scalar_tensor_tensor(
                    out=x[:, m * D + 512:(m + 1) * D], in0=ps3b[:, 0:256], scalar=0.0,
                    in1=qrow[:, 512:D],
                    op0=ALU.add, op1=ALU.add, accum_out=s1b[:])
                nc.vector.tensor_tensor(out=s1[:], in0=s1a[:], in1=s1b[:], op=ALU.add)
                mun = sp.tile([128, 1], dt.float32, name="mun_t", tag="mun_t", bufs=4)
                nc.vector.tensor_scalar_mul(mun[:], s1[:], -1.0 / D)
                ssq = sp.tile([128, 1], dt.float32, name="ssq_t", tag="ssq_t", bufs=4)
                nc.scalar.activation(scratch[:], xm, ACTF.Square,
                                     bias=mun[:], scale=1.0, accum_out=ssq[:])
                std = sp.tile([128, 1], dt.float32, name="std_t", tag="std_t", bufs=4)
                nc.scalar.activation(std[:], ssq[:], ACTF.Sqrt,
                                     bias=eps_col[:], scale=1.0 / D)
                rstd = sp.tile([128, 1], dt.float32, name="rstd_t", tag="rstd_t", bufs=4)
                nc.vector.reciprocal_approx_fast(out=rstd[:], in_=std[:])
                nc.vector.tensor_scalar(out=xm, in0=xm, scalar1=mun[:], scalar2=rstd[:],
                                        op0=ALU.add, op1=ALU.mult)
            nc.sync.dma_start(out_o[b].rearrange("(m p) e -> p m e", p=128),
                              x[:].rearrange("p (m e) -> p m e", m=MC_L))

    nc.compile()
    globals()['BC'] = BC_saved
    return nc


def _get_nc():
    if "nc" not in _CACHE:
        _CACHE["nc"] = _build_program()
    return _CACHE["nc"]


def _prep_inputs(q, k, v, Wq, Wk, Wv, Wconv, bconv, Wfc, mask1, mask2):
    temp = np.sqrt(np.float32(DK))
    wq = (np.asarray(Wq, np.float32) / temp).astype(bf16)
    wk = np.asarray(Wk, np.float32).astype(bf16)
    wv = np.asarray(Wv, np.float32).astype(bf16)
    wct = np.ascontiguousarray(np.asarray(Wconv, np.float32).T).astype(bf16)
    wfc = np.asarray(Wfc, np.float32).astype(bf16)
    bq = (np.asarray(bconv, np.float32) / temp).astype(bf16).reshape(1, L)
    bk = np.asarray(bconv, np.float32).astype(bf16).reshape(1, L)
    c1 = (1.0 - np.asarray(mask1, np.float32))
    c2 = (1.0 - np.asarray(mask2, np.float32))
    q = np.asarray(q, np.float32)
    k = np.asarray(k, np.float32)
    v = np.asarray(v, np.float32)
    qT = np.ascontiguousarray(q.transpose(0, 2, 1)).astype(bf16)
    kT = np.ascontiguousarray(k.transpose(0, 2, 1)).astype(bf16)
    vT = np.ascontiguousarray(v.transpose(0, 2, 1)).astype(bf16)

    in_maps = []
    for c in range(NCORES):
        s = slice(c * BC, (c + 1) * BC)
        in_maps.append({
            "qT": qT[s], "kT": kT[s], "vT": vT[s],
            "qrow": np.ascontiguousarray(q[s]),
            "wq": wq, "wk": wk, "wv": wv, "wct": wct, "wfc": wfc,
            "bq": bq, "bk": bk,
            "c1": np.ascontiguousarray(c1[s].reshape(1, BC * L)).astype(bf16),
            "c2": np.ascontiguousarray(c2[s].reshape(1, BC * L)).astype(bf16),
        })
    return in_maps


def _run(in_maps, trace=False):
    from concourse.bass_utils import run_bass_kernel_spmd
    nc = _get_nc()
    res = run_bass_kernel_spmd(nc, in_maps, list(range(NCORES)), trace=trace)
    return res


def _gather(res, gamma, beta):
    out = np.empty((B, L, D), np.float32)
    attn = np.empty((B, H, L, L), np.float32)
    for c in range(NCORES):
        r = res.results[c]
        sums = np.asarray(r["sums_o"], np.float32).reshape(BC, H, L)
        expT = np.asarray(r["expT_o"]).astype(np.float32)       # (BC,H,L,L) [k,q]
        attn[c * BC:(c + 1) * BC] = expT.transpose(0, 1, 3, 2) / sums[:, :, :, None]
        out[c * BC:(c + 1) * BC] = r["out_o"]
    out = out * np.asarray(gamma, np.float32) + np.asarray(beta, np.float32)
    out = np.where(np.isnan(out), np.float32(0), out)
    attn = np.where(np.isnan(attn), np.float32(0), attn)
    return out, attn


def kernel(q, k, v, Wq, Wk, Wv, Wconv, bconv, Wfc, gamma, beta, mask1, mask2):
    in_maps = _prep_inputs(q, k, v, Wq, Wk, Wv, Wconv, bconv, Wfc, mask1, mask2)
    res = _run(in_maps, trace=False)
    return _gather(res, gamma, beta)


# revision 16
# speedup vs baseline: 1.1824x; 1.0054x over previous
"""Trainium2 Bass kernel for nn_MultiHeadAttention_523986010579.

Strategy: data-parallel over batch (B=32) across 8 NeuronCores (4 batches
per core).  Inside each core everything is computed with bf16 matmul
operands / fp32 PSUM accumulation.

Per-core dataflow (per batch b):
  qT,kT,vT fed pre-transposed (d_emb on partitions) from host.
  q1  = qT.T @ Wq'            (row-major, L on partitions; Wq' = Wq/sqrt(Dk))
  k1  = kT.T @ Wk
  v1  = vT.T @ Wv             (row-major)
  q1cT[dk,o] = (q1.T @ Wconv.T + bconv') * (1-mask1)[o]   (bias via K=1 matmul,
  k1cT[dk,o] = (k1.T @ Wconv.T + bconv ) * (1-mask2)[o]    mask folded at evac)
  per head h:
    logitsT[k,q] = k1cT_h.T @ q1cT_h          (K=64 contraction)
    expT = exp(logitsT)                        (no max-sub: |logits| < 0.2)
    sums[q] = ones.T @ expT                    (PE ones-trick)
    out2T[dv,q] = (v1_h.T @ expT) * (1/sums)[q]  (recip broadcast via K=1 matmul)
  out3 = out2T.T @ Wfc + q                    (residual, fp32)
  LayerNorm(out3) on the free axis            (fp32)
Outputs: unnormalized expT (bf16) + sums (f32) + normalized out (f32).
Host: attn = expT.T / sums, out = out*gamma+beta, nan->0, gather shards.

The mask fill value 1e-9 in the reference is numerically ~0, so masking
is exactly a {0,1} row/col scale of the logits (error ~1e-9, far below
fp32 noise).
"""

import numpy as np
import ml_dtypes

B, L, D, DK, DV, H = 32, 512, 768, 512, 1024, 8
NCORES = 8
BC = B // NCORES          # batches per core
KC_D = D // 128           # 6  k-chunks of d_emb
MC_L = L // 128           # 4  m-chunks of L
KC_L = L // 128           # 4  k-chunks of L (conv contraction / attn keys)
MC_DK = DK // 128         # 4  m-chunks of d_k
HC_DV = DV // 128         # 8  chunks of d_v (== heads, dh_v = 128)
DH_K = DK // H            # 64
LN_EPS = 1e-6

bf16 = ml_dtypes.bfloat16

_CACHE = {}


def _build_program(bc=BC):
    import concourse.bass as bass
    import concourse.bacc as bacc
    import concourse.tile as tile
    import concourse.mybir as mybir
    from contextlib import ExitStack

    dt = mybir.dt
    ALU = mybir.AluOpType
    ACTF = mybir.ActivationFunctionType

    global BC
    BC_saved = BC
    BC = bc
    nc = bacc.Bacc("TRN2", target_bir_lowering=False, debug=False)

    # ---- DRAM I/O (per-core shard shapes) ----
    din = {}
    def dram_in(name, shape, dtype):
        din[name] = nc.dram_tensor(name, list(shape), dtype, kind="ExternalInput").ap()
        return din[name]

    qT_d = dram_in("qT", (BC, D, L), dt.bfloat16)
    kT_d = dram_in("kT", (BC, D, L), dt.bfloat16)
    vT_d = dram_in("vT", (BC, D, L), dt.bfloat16)
    qrow_d = dram_in("qrow", (BC, L, D), dt.float32)
    wq_d = dram_in("wq", (D, DK), dt.bfloat16)      # pre-scaled by 1/sqrt(DK)
    wk_d = dram_in("wk", (D, DK), dt.bfloat16)
    wv_d = dram_in("wv", (D, DV), dt.bfloat16)
    wct_d = dram_in("wct", (L, L), dt.bfloat16)     # Wconv.T
    wfc_d = dram_in("wfc", (DV, D), dt.bfloat16)
    bq_d = dram_in("bq", (1, L), dt.bfloat16)       # bconv/sqrt(DK)
    bk_d = dram_in("bk", (1, L), dt.bfloat16)       # bconv
    c1_d = dram_in("c1", (1, BC * L), dt.bfloat16)   # 1-mask1 per batch
    c2_d = dram_in("c2", (1, BC * L), dt.bfloat16)   # 1-mask2 per batch

    expT_o = nc.dram_tensor("expT_o", [BC, H, L, L], dt.bfloat16, kind="ExternalOutput").ap()
    sums_o = nc.dram_tensor("sums_o", [BC, H * L], dt.float32, kind="ExternalOutput").ap()
    out_o = nc.dram_tensor("out_o", [BC, L, D], dt.float32, kind="ExternalOutput").ap()

    with tile.TileContext(nc) as tc, ExitStack() as ctx:
        wp = ctx.enter_context(tc.tile_pool(name="wp", bufs=1))
        io = ctx.enter_context(tc.tile_pool(name="io", bufs=2))
        ip = ctx.enter_context(tc.tile_pool(name="ip", bufs=2))
        sp = ctx.enter_context(tc.tile_pool(name="sp", bufs=4))
        pp = ctx.enter_context(tc.tile_pool(name="pp", bufs=5, space="PSUM"))

        # ---- weights / constants (once) ----
        wq = wp.tile([128, KC_D * DK], dt.bfloat16, name="wq_t", tag="wq_t")
        nc.sync.dma_start(wq[:].rearrange("p (kc j) -> p kc j", kc=KC_D),
                          wq_d.rearrange("(kc p) j -> p kc j", p=128))
        wk = wp.tile([128, KC_D * DK], dt.bfloat16, name="wk_t", tag="wk_t")
        nc.sync.dma_start(wk[:].rearrange("p (kc j) -> p kc j", kc=KC_D),
                          wk_d.rearrange("(kc p) j -> p kc j", p=128))
        wv = wp.tile([128, KC_D * DV], dt.bfloat16, name="wv_t", tag="wv_t")
        nc.sync.dma_start(wv[:].rearrange("p (kc j) -> p kc j", kc=KC_D),
                          wv_d.rearrange("(kc p) j -> p kc j", p=128))
        bq = wp.tile([1, L], dt.bfloat16, name="bq_t", tag="bq_t")
        nc.sync.dma_start(bq[:], bq_d)
        bk = wp.tile([1, L], dt.bfloat16, name="bk_t", tag="bk_t")
        nc.sync.dma_start(bk[:], bk_d)
        c1a = wp.tile([1, BC * L], dt.bfloat16, name="c1a_t", tag="c1a_t")
        nc.sync.dma_start(c1a[:], c1_d)
        c2a = wp.tile([1, BC * L], dt.bfloat16, name="c2a_t", tag="c2a_t")
        nc.sync.dma_start(c2a[:], c2_d)
        ones_col = wp.tile([128, 1], dt.bfloat16, name="ones_col", tag="ones_col")
        nc.vector.memset(ones_col[:], 1.0)
        ones_row = wp.tile([1, 128], dt.bfloat16, name="ones_row", tag="ones_row")
        nc.vector.memset(ones_row[:], 1.0)
        eps_col = wp.tile([128, 1], dt.float32, name="eps_col", tag="eps_col")
        nc.vector.memset(eps_col[:], float(LN_EPS))

        wct = wfc = None
        for b in range(BC):
            # ---- load activations ----
            qT = io.tile([128, KC_D * L], dt.bfloat16, name="qT_t", tag="qT_t")
            nc.sync.dma_start(qT[:].rearrange("p (kc j) -> p kc j", kc=KC_D),
                              qT_d[b].rearrange("(kc p) j -> p kc j", p=128))
            kT = io.tile([128, KC_D * L], dt.bfloat16, name="kT_t", tag="kT_t")
            nc.sync.dma_start(kT[:].rearrange("p (kc j) -> p kc j", kc=KC_D),
                              kT_d[b].rearrange("(kc p) j -> p kc j", p=128))
            vT = io.tile([128, KC_D * L], dt.bfloat16, name="vT_t", tag="vT_t")
            nc.sync.dma_start(vT[:].rearrange("p (kc j) -> p kc j", kc=KC_D),
                              vT_d[b].rearrange("(kc p) j -> p kc j", p=128))

            if wct is None:
                wct = wp.tile([128, KC_L * L], dt.bfloat16, name="wct_t", tag="wct_t")
                nc.sync.dma_start(wct[:].rearrange("p (kc j) -> p kc j", kc=KC_L),
                                  wct_d.rearrange("(kc p) j -> p kc j", p=128))
                wfc = wp.tile([128, HC_DV * D], dt.bfloat16, name="wfc_t", tag="wfc_t")
                nc.sync.dma_start(wfc[:].rearrange("p (kc j) -> p kc j", kc=HC_DV),
                                  wfc_d.rearrange("(kc p) j -> p kc j", p=128))

            # ---- mask broadcast tiles: ones_row.T @ c_row ----
            c1b = ip.tile([128, L], dt.float32, name="c1b_t", tag="c1b_t", bufs=1)
            c2b = ip.tile([128, L], dt.float32, name="c2b_t", tag="c2b_t", bufs=1)
            for cb, ca in ((c1b, c1a), (c2b, c2a)):
                ps = pp.tile([128, 512], dt.float32, name="ps_cb", tag="pS", bufs=3)
                nc.tensor.matmul(ps[:], ones_row[:], ca[0:1, b * L:(b + 1) * L],
                                 start=True, stop=True)
                nc.scalar.copy(cb[:], ps[:])

            # ---- projections: q1/k1 row-major ----
            q1 = ip.tile([128, MC_L * DK], dt.bfloat16, name="q1_t", tag="q1_t", bufs=1)
            k1 = ip.tile([128, MC_L * DK], dt.bfloat16, name="k1_t", tag="k1_t", bufs=1)
            for dst, src, w in ((q1, qT, wq), (k1, kT, wk)):
                for m in range(MC_L):
                    ps = pp.tile([128, 512], dt.float32, name="ps_p", tag="pA")
                    for kc in range(KC_D):
                        nc.tensor.matmul(
                            ps[:],
                            src[:, kc * L + m * 128: kc * L + (m + 1) * 128],
                            w[:, kc * DK:(kc + 1) * DK],
                            start=(kc == 0), stop=(kc == KC_D - 1))
                    nc.vector.tensor_copy(dst[:, m * DK:(m + 1) * DK], ps[:])

            # ---- v1 row-major (dv=1024 -> two 512 psums per m-chunk) ----
            v1 = ip.tile([128, MC_L * DV], dt.bfloat16, name="v1_t", tag="v1_t", bufs=2)
            for m in range(MC_L):
                for half in range(2):
                    ps = pp.tile([128, 512], dt.float32, name="ps_v", tag="pA")
                    for kc in range(KC_D):
                        nc.tensor.matmul(
                            ps[:],
                            vT[:, kc * L + m * 128: kc * L + (m + 1) * 128],
                            wv[:, kc * DV + half * 512: kc * DV + (half + 1) * 512],
                            start=(kc == 0), stop=(kc == KC_D - 1))
                    nc.scalar.copy(v1[:, m * DV + half * 512: m * DV + (half + 1) * 512], ps[:])

            # ---- conv(+bias) then mask at evac -> q1cT / k1cT ----
            q1cT = ip.tile([128, MC_DK * L], dt.bfloat16, name="q1cT_t", tag="q1cT_t")
            k1cT = ip.tile([128, MC_DK * L], dt.bfloat16, name="k1cT_t", tag="k1cT_t")
            for dst, src, bias, cb in ((q1cT, q1, bq, c1b), (k1cT, k1, bk, c2b)):
                for m in range(MC_DK):
                    ps = pp.tile([128, 512], dt.float32, name="ps_c", tag="pA")
                    for kc in range(KC_L):
                        nc.tensor.matmul(
                            ps[:],
                            src[:, kc * DK + m * 128: kc * DK + (m + 1) * 128],
                            wct[:, kc * L:(kc + 1) * L],
                            start=(kc == 0), stop=False)
                    nc.tensor.matmul(ps[:], ones_row[:], bias[:], start=False, stop=True)
                    nc.vector.tensor_tensor(
                        out=dst[:, m * L:(m + 1) * L], in0=ps[:], in1=cb[:], op=ALU.mult)

            # ---- attention heads ----
            o2T = ip.tile([128, HC_DV * L], dt.bfloat16, name="o2T_t", tag="o2T_t", bufs=2)
            expT_pair = [None, None]
            for h in range(H):
                jc = h // 2
                d0 = (h % 2) * DH_K
                if h % 2 == 0:
                    # compute logits for BOTH heads of the pair with adjacent
                    # row-group-packed matmuls (K=64 each, concurrent in PE)
                    e0 = sp.tile([128, KC_L * L], dt.bfloat16, name="expT_t", tag="expT_t", bufs=4)
                    e1 = sp.tile([128, KC_L * L], dt.bfloat16, name="expT2_t", tag="expT2_t", bufs=2)
                    expT_pair = [e0, e1]
                    for kc in range(KC_L):
                        ps_l0 = pp.tile([128, 512], dt.float32, name="ps_l0", tag="pA")
                        ps_l1 = pp.tile([128, 512], dt.float32, name="ps_l1", tag="pA")
                        nc.tensor.matmul(
                            ps_l0[:],
                            k1cT[0:DH_K, jc * L + kc * 128: jc * L + (kc + 1) * 128],
                            q1cT[0:DH_K, jc * L:(jc + 1) * L],
                            start=True, stop=True, tile_position=(0, 0))
                        nc.tensor.matmul(
                            ps_l1[:],
                            k1cT[DH_K:2 * DH_K, jc * L + kc * 128: jc * L + (kc + 1) * 128],
                            q1cT[DH_K:2 * DH_K, jc * L:(jc + 1) * L],
                            start=True, stop=True, tile_position=(64, 0))
                        nc.scalar.activation(e0[:, kc * L:(kc + 1) * L], ps_l0[:], ACTF.Exp)
                        nc.scalar.activation(e1[:, kc * L:(kc + 1) * L], ps_l1[:], ACTF.Exp)
                expT = expT_pair[h % 2]
                ps_s = pp.tile([1, 512], dt.float32, name="ps_s", tag="pS", bufs=3)
                for kc in range(KC_L):
                    nc.tensor.matmul(ps_s[:], ones_col[:], expT[:, kc * L:(kc + 1) * L],
                                     start=(kc == 0), stop=(kc == KC_L - 1))
                # out2T_unnorm = v1_h.T @ expT  (keeps PE off the recip path)
                ps_o2 = pp.tile([128, 512], dt.float32, name="ps_o2", tag="pS", bufs=3)
                for kc in range(KC_L):
                    nc.tensor.matmul(
                        ps_o2[:],
                        v1[:, kc * DV + h * 128: kc * DV + (h + 1) * 128],
                        expT[:, kc * L:(kc + 1) * L],
                        start=(kc == 0), stop=(kc == KC_L - 1))
                sums_h = sp.tile([1, L], dt.float32, name="sums_h", tag="sums_h", bufs=2)
                nc.vector.tensor_copy(sums_h[:], ps_s[:])
                nc.sync.dma_start(sums_o[b:b + 1, h * L:(h + 1) * L], sums_h[:])
                nc.sync.dma_start(
                    expT_o[b, h].rearrange("(kc p) q -> p kc q", p=128),
                    expT[:].rearrange("p (kc q) -> p kc q", kc=KC_L))
                # reciprocal row -> broadcast tile (PE rb matmul is cheap, off critical path)
                recf = sp.tile([1, L], dt.float32, name="recf_t", tag="recf_t", bufs=2)
                nc.vector.reciprocal_approx_fast(out=recf[:], in_=ps_s[:])
                rb = sp.tile([128, L], dt.float32, name="rb_t", tag="rb_t", bufs=2)
                nc.gpsimd.partition_broadcast(rb[:], recf[:])
                nc.vector.tensor_tensor(out=o2T[:, h * L:(h + 1) * L],
                                        in0=ps_o2[:], in1=rb[:], op=ALU.mult)

            # ---- fc + residual + LayerNorm ----
            x = ip.tile([128, MC_L * D], dt.float32, name="x_t", tag="x_t", bufs=1)
            scratch = ip.tile([128, D], dt.bfloat16, name="scr_t", tag="scr_t", bufs=2)
            for m in range(MC_L):
                qrow = io.tile([128, D], dt.float32, name="qrow_t", tag="qrow_t", bufs=2)
                nc.sync.dma_start(qrow[:], qrow_d[b, m * 128:(m + 1) * 128, :])
                ps3a = pp.tile([128, 512], dt.float32, name="ps3a", tag="pA")
                ps3b = pp.tile([128, 512], dt.float32, name="ps3b", tag="pA")
                for hc in range(HC_DV):
                    nc.tensor.matmul(
                        ps3a[:],
                        o2T[:, hc * L + m * 128: hc * L + (m + 1) * 128],
                        wfc[:, hc * D: hc * D + 512],
                        start=(hc == 0), stop=(hc == HC_DV - 1))
                for hc in range(HC_DV):
                    nc.tensor.matmul(
                        ps3b[:, 0:256],
                        o2T[:, hc * L + m * 128: hc * L + (m + 1) * 128],
                        wfc[:, hc * D + 512: hc * D + D],
                        start=(hc == 0), stop=(hc == HC_DV - 1))
                s1a = sp.tile([128, 1], dt.float32, name="s1a_t", tag="s1a_t", bufs=4)
                s1b = sp.tile([128, 1], dt.float32, name="s1b_t", tag="s1b_t", bufs=4)
                s1 = sp.tile([128, 1], dt.float32, name="s1_t", tag="s1_t", bufs=4)
                xm = x[:, m * D:(m + 1) * D]
                nc.vector.scalar_tensor_tensor(
                    out=x[:, m * D: m * D + 512], in0=ps3a[:], scalar=0.0,
                    in1=qrow[:, 0:512],
                    op0=ALU.add, op1=ALU.add, accum_out=s1a[:])
                nc.vector.scalar_tensor_tensor(
                    out=x[:, m * D + 512:(m + 1) * D], in0=ps3b[:, 0:256], scalar=0.0,
                    in1=qrow[:, 512:D],
                    op0=ALU.add, op1=ALU.add, accum_out=s1b[:])
                nc.vector.tensor_tensor(out=s1[:], in0=s1a[:], in1=s1b[:], op=ALU.add)
                mun = sp.tile([128, 1], dt.float32, name="mun_t", tag="mun_t", bufs=4)
                nc.vector.tensor_scalar_mul(mun[:], s1[:], -1.0 / D)
                ssq = sp.tile([128, 1], dt.float32, name="ssq_t", tag="ssq_t", bufs=4)
                nc.scalar.activation(scratch[:], xm, ACTF.Square,
                                     bias=mun[:], scale=1.0, accum_out=ssq[:])
                std = sp.tile([128, 1], dt.float32, name="std_t", tag="std_t", bufs=4)
                nc.scalar.activation(std[:], ssq[:], ACTF.Sqrt,
                                     bias=eps_col[:], scale=1.0 / D)
                rstd = sp.tile([128, 1], dt.float32, name="rstd_t", tag="rstd_t", bufs=4)
                nc.vector.reciprocal_approx_fast(out=rstd[:], in_=std[:])
                nc.vector.tensor_scalar(out=xm, in0=xm, scalar1=mun[:], scalar2=rstd[:],
                                        op0=ALU.add, op1=ALU.mult)
            nc.sync.dma_start(out_o[b].rearrange("(m p) e -> p m e", p=128),
                              x[:].rearrange("p (m e) -> p m e", m=MC_L))

    nc.compile()
    globals()['BC'] = BC_saved
    return nc


def _get_nc():
    if "nc" not in _CACHE:
        _CACHE["nc"] = _build_program()
    return _CACHE["nc"]


def _prep_inputs(q, k, v, Wq, Wk, Wv, Wconv, bconv, Wfc, mask1, mask2):
    temp = np.sqrt(np.float32(DK))
    wq = (np.asarray(Wq, np.float32) / temp).astype(bf16)
    wk = np.asarray(Wk, np.float32).astype(bf16)
    wv = np.asarray(Wv, np.float32).astype(bf16)
    wct = np.ascontiguousarray(np.asarray(Wconv, np.float32).T).astype(bf16)
    wfc = np.asarray(Wfc, np.float32).astype(bf16)
    bq = (np.asarray(bconv, np.float32) / temp).astype(bf16).reshape(1, L)
    bk = np.asarray(bconv, np.float32).astype(bf16).reshape(1, L)
    c1 = (1.0 - np.asarray(mask1, np.float32))
    c2 = (1.0 - np.asarray(mask2, np.float32))
    q = np.asarray(q, np.float32)
    k = np.asarray(k, np.float32)
    v = np.asarray(v, np.float32)
    qT = np.ascontiguousarray(q.transpose(0, 2, 1)).astype(bf16)
    kT = np.ascontiguousarray(k.transpose(0, 2, 1)).astype(bf16)
    vT = np.ascontiguousarray(v.transpose(0, 2, 1)).astype(bf16)

    in_maps = []
    for c in range(NCORES):
        s = slice(c * BC, (c + 1) * BC)
        in_maps.append({
            "qT": qT[s], "kT": kT[s], "vT": vT[s],
            "qrow": np.ascontiguousarray(q[s]),
            "wq": wq, "wk": wk, "wv": wv, "wct": wct, "wfc": wfc,
            "bq": bq, "bk": bk,
            "c1": np.ascontiguousarray(c1[s].reshape(1, BC * L)).astype(bf16),
            "c2": np.ascontiguousarray(c2[s].reshape(1, BC * L)).astype(bf16),
        })
    return in_maps


def _run(in_maps, trace=False):
    from concourse.bass_utils import run_bass_kernel_spmd
    nc = _get_nc()
    res = run_bass_kernel_spmd(nc, in_maps, list(range(NCORES)), trace=trace)
    return res


def _gather(res, gamma, beta):
    out = np.empty((B, L, D), np.float32)
    attn = np.empty((B, H, L, L), np.float32)
    for c in range(NCORES):
        r = res.results[c]
        sums = np.asarray(r["sums_o"], np.float32).reshape(BC, H, L)
        expT = np.asarray(r["expT_o"]).astype(np.float32)       # (BC,H,L,L) [k,q]
        attn[c * BC:(c + 1) * BC] = expT.transpose(0, 1, 3, 2) / sums[:, :, :, None]
        out[c * BC:(c + 1) * BC] = r["out_o"]
    out = out * np.asarray(gamma, np.float32) + np.asarray(beta, np.float32)
    out = np.where(np.isnan(out), np.float32(0), out)
    attn = np.where(np.isnan(attn), np.float32(0), attn)
    return out, attn


def kernel(q, k, v, Wq, Wk, Wv, Wconv, bconv, Wfc, gamma, beta, mask1, mask2):
    in_maps = _prep_inputs(q, k, v, Wq, Wk, Wv, Wconv, bconv, Wfc, mask1, mask2)
    res = _run(in_maps, trace=False)
    return _gather(res, gamma, beta)
